# revision 14
# baseline (speedup 1.0000x reference)
"""Sharded kNN (cosine retrieval) kernel for 8 Trainium2 NeuronCores.

Strategy
--------
Shard the memory bank (mem_descriptors, rank) across the 8 cores along N.
Host prep (cheap, O(N*F)): normalize rows of both matrices, zero out rows with
rank<=0, pad each 62500-row shard to 65536, transpose to [F, N_loc], and cast
to fp8-e4m3 in the DoubleRow-packed layout [128, 2, N] (k and k+128 paired).

Device (per core): score matrix S = x_hat^T-block @ y_hat tiles on the
TensorEngine (fp8 DoubleRow: one matmul per 512 columns does the full K=256
contraction at ~1.4x bf16 FLOP rate), accumulated in fp32 PSUM as [128, 2048]
four-bank groups.  Each group is max-folded into [128, 2048] running-max
accumulators (acc[j] = max over columns n == j mod 2048).  Fold work is split:
VectorE folds ~1/4 of the groups directly (fp32 from PSUM), ScalarE copies the
other ~3/4 to bf16 SBUF (with a +1.0 bias) and VectorE folds those at its 2x
bf16 rate.  Each accumulator is then folded 2048->512 and finished with one
top-8 max + max_index (positions are column mod 512).

Host finish (tiny): expand each near-max folded position into its candidate
columns, rescore those few candidates exactly in fp32 against the original
data, and emit (cos_max, argmax index, gathered descriptor) matching the
reference semantics.  Correct because the true winner's device score is within
~7e-3 of the device max (fp8 rounding, measured on the real inputs), inside
the 0.02 margin, and every row's global max is positive (so zeroed
invalid/padded entries never win).
"""

import numpy as np
import ml_dtypes

M = 512          # queries
F = 256          # feature dim
N = 500000       # memory rows
NCORES = 8
N_PER = N // NCORES          # 62500
N_PAD = 65536                # per-core padded columns
ACC_W = 2048                 # running accumulator width
IDX_W = 512                  # final fold width (positions are mod IDX_W)
N_BLOCKS = 16                # n-blocks of 4096 columns
BLK = 4096
GW = 2048                    # psum group width (4 banks)
PT = 512                     # single matmul output width
MT = M // 128                # 4 query blocks
KT = F // 128                # 2 contraction tiles (packed into DoubleRow pairs)
MARGIN = 0.02

# fold-engine pattern over psum groups:
# V = direct DVE fold (fp32 from PSUM), A = ACT copy (+bias, bf16) + DVE 2x fold
FOLD_PATTERN = ["A", "A", "V", "A"]
BIAS = 1.0

_CACHE = {}


def _build_bass():
    import concourse.mybir as mybir
    import concourse.tile as tile
    from concourse import bacc

    fp8 = mybir.dt.float8e4
    bf16 = mybir.dt.bfloat16
    f32 = mybir.dt.float32
    u32 = mybir.dt.uint32
    DR = mybir.MatmulPerfMode.DoubleRow

    nc = bacc.Bacc("TRN2", target_bir_lowering=False, debug=False,
                   num_devices=NCORES)

    xt = nc.dram_tensor("xt", [128, KT, M], fp8, kind="ExternalInput").ap()
    yt = nc.dram_tensor("yt", [128, KT, N_PAD], fp8, kind="ExternalInput").ap()
    # 16 = top-8 from the fp32 acc + top-8 from the bf16 acc
    vals = nc.dram_tensor("vals", [M, 16], f32, kind="ExternalOutput").ap()
    idxs = nc.dram_tensor("idxs", [M, 16], u32, kind="ExternalOutput").ap()

    with tile.TileContext(nc) as tc:
        with (
            tc.tile_pool(name="xpool", bufs=1) as xpool,
            tc.tile_pool(name="ypool", bufs=3) as ypool,
            tc.tile_pool(name="apool", bufs=1) as apool,
            tc.tile_pool(name="spool", bufs=4) as spool,
            tc.tile_pool(name="opool", bufs=4) as opool,
            tc.tile_pool(name="psum", bufs=2, space="PSUM") as pspool,
        ):
            xk = xpool.tile([128, KT, M], fp8, tag="x")
            nc.sync.dma_start(out=xk[:], in_=xt[:])

            acc_v, acc_g = [], []
            for m in range(MT):
                av = apool.tile([128, ACC_W], f32, tag=f"accv{m}")
                nc.gpsimd.memset(av[:], -1e30)
                acc_v.append(av)
                ag = apool.tile([128, ACC_W], bf16, tag=f"accg{m}")
                nc.gpsimd.memset(ag[:], 0.0)   # biased scores are all >= 0.7
                acc_g.append(ag)

            cnt = [0] * MT  # per-m group counters, drive the fold pattern
            for b in range(N_BLOCKS):
                yk = ypool.tile([128, KT, BLK], fp8, tag="y")
                nc.sync.dma_start(out=yk[:],
                                  in_=yt[:, :, b * BLK:(b + 1) * BLK])
                for m in range(MT):
                    for grp in range(BLK // GW):
                        ps = pspool.tile([128, GW], f32, tag="ps")
                        for sub in range(GW // PT):
                            c0 = grp * GW + sub * PT
                            nc.tensor.matmul(
                                ps[:, sub * PT:(sub + 1) * PT],
                                xk[:, :, m * 128:(m + 1) * 128],
                                yk[:, :, c0:c0 + PT],
                                start=True, stop=True, perf_mode=DR)
                        gg = cnt[m]
                        cnt[m] += 1
                        if FOLD_PATTERN[gg % len(FOLD_PATTERN)] == "V":
                            dst = acc_v[m][:]
                            nc.vector.tensor_max(dst, dst, ps[:])
                        else:
                            stg = spool.tile([128, GW], bf16, tag="stg")
                            nc.scalar.add(stg[:], ps[:], BIAS)
                            dst = acc_g[m][:]
                            nc.vector.tensor_max(dst, dst, stg[:])

            for m in range(MT):
                # pre-fold each accumulator ACC_W -> IDX_W, then top-8
                # (positions become column mod IDX_W)
                fv = opool.tile([128, 1024], f32, tag="fv")
                nc.vector.tensor_max(fv[:], acc_v[m][:, 0:1024],
                                     acc_v[m][:, 1024:2048])
                nc.vector.tensor_max(fv[:, 0:512], fv[:, 0:512],
                                     fv[:, 512:1024])
                t8 = opool.tile([128, 8], f32, tag="t8")
                i8 = opool.tile([128, 8], u32, tag="i8")
                nc.vector.max(t8[:], fv[:, 0:IDX_W])
                nc.vector.max_index(i8[:], t8[:], fv[:, 0:IDX_W])
                nc.sync.dma_start(out=vals[m * 128:(m + 1) * 128, 0:8],
                                  in_=t8[:])
                nc.sync.dma_start(out=idxs[m * 128:(m + 1) * 128, 0:8],
                                  in_=i8[:])

                fg = opool.tile([128, 1024], bf16, tag="fg")
                nc.vector.tensor_max(fg[:], acc_g[m][:, 0:1024],
                                     acc_g[m][:, 1024:2048])
                nc.vector.tensor_max(fg[:, 0:512], fg[:, 0:512],
                                     fg[:, 512:1024])
                t8g = opool.tile([128, 8], bf16, tag="t8g")
                t8gf = opool.tile([128, 8], f32, tag="t8gf")
                i8g = opool.tile([128, 8], u32, tag="i8g")
                nc.vector.max(t8g[:], fg[:, 0:IDX_W])
                nc.vector.max_index(i8g[:], t8g[:], fg[:, 0:IDX_W])
                # un-bias while converting bf16 -> f32
                nc.vector.tensor_scalar_add(t8gf[:], t8g[:], -BIAS)
                nc.sync.dma_start(out=vals[m * 128:(m + 1) * 128, 8:16],
                                  in_=t8gf[:])
                nc.sync.dma_start(out=idxs[m * 128:(m + 1) * 128, 8:16],
                                  in_=i8g[:])

    nc.compile()
    return nc


def _get_nc():
    if "nc" not in _CACHE:
        _CACHE["nc"] = _build_bass()
    return _CACHE["nc"]


def run_device(in_maps, trace=False):
    """Run the compiled SPMD kernel; returns (results, BassKernelResults)."""
    from concourse.bass_utils import run_bass_kernel_spmd
    nc = _get_nc()
    r = run_bass_kernel_spmd(nc, in_maps, core_ids=list(range(NCORES)),
                             trace=trace)
    return r.results, r


def prep_inputs(descriptors, mem_descriptors, rank):
    """Host-side normalization / sharding / layout prep."""
    x = np.asarray(descriptors, np.float32)
    y = np.asarray(mem_descriptors, np.float32)
    r = np.asarray(rank, np.float32)

    xx = np.linalg.norm(x, axis=1, keepdims=True)          # [M,1]
    yy = np.linalg.norm(y, axis=1)                          # [N]
    xhat = (x / np.maximum(xx, 1e-30)).astype(np.float32)
    scale = np.where(r > 0, 1.0 / np.maximum(yy, 1e-30), 0.0).astype(np.float32)

    fp8 = ml_dtypes.float8_e4m3
    # DoubleRow layout: [p, j, col] = mat[col, 128*j + p]
    xt = np.ascontiguousarray(
        xhat.T.reshape(KT, 128, M).transpose(1, 0, 2)).astype(fp8)

    in_maps = []
    for c in range(NCORES):
        sh = y[c * N_PER:(c + 1) * N_PER]                   # [N_PER, F]
        sc = scale[c * N_PER:(c + 1) * N_PER]
        ytc = np.zeros((128, KT, N_PAD), dtype=fp8)
        yh = (sh * sc[:, None]).T                           # [F, N_PER] f32
        ytc[:, :, :N_PER] = yh.reshape(KT, 128, N_PER).transpose(1, 0, 2).astype(fp8)
        in_maps.append({"xt": xt, "yt": ytc})
    return in_maps, x, y, r, xx[:, 0], yy


def finish(results, x, y, r, xx, yy):
    """Expand device candidates, rescore exactly, emit reference-equal output."""
    vals = np.stack([np.asarray(results[c]["vals"], np.float32)
                     for c in range(NCORES)])               # [8, M, 16]
    idxs = np.stack([np.asarray(results[c]["idxs"], np.int64)
                     for c in range(NCORES)])               # [8, M, 16]

    gmax = vals.max(axis=(0, 2))                            # [M]
    keep = vals >= (gmax[None, :, None] - MARGIN)           # [8, M, 16]

    slices = np.arange(N_PAD // IDX_W) * IDX_W              # fold slices

    cos_max = np.empty(M, np.float32)
    best_idx = np.empty(M, np.int64)
    valid = r > 0
    for m in range(M):
        cs, ss = np.nonzero(keep[:, m, :])
        cand = (idxs[cs, m, ss][:, None] + slices[None, :]).ravel()
        cores = np.repeat(cs, slices.size)
        ok = cand < N_PER
        ng = cores[ok] * N_PER + cand[ok]
        ng = np.unique(ng)
        ng = ng[valid[ng]]
        xy = y[ng] @ x[m]                                   # fp32 exact
        cos = xy / np.maximum(xx[m] * yy[ng], np.float32(1e-7))
        j = int(np.argmax(cos))
        ties = np.nonzero(cos == cos[j])[0]
        j = int(ties[np.argmin(ng[ties])])
        cos_max[m] = cos[j]
        best_idx[m] = ng[j]

    out_desc = y[best_idx]
    return (cos_max.astype(np.float32), best_idx.astype(np.int32),
            out_desc.astype(np.float32))


def kernel(descriptors, mem_descriptors, rank):
    in_maps, x, y, r, xx, yy = prep_inputs(descriptors, mem_descriptors, rank)
    results, _ = run_device(in_maps)
    return finish(results, x, y, r, xx, yy)


# revision 17
# speedup vs baseline: 1.1235x; 1.1235x over previous
"""Sharded kNN (cosine retrieval) kernel for 8 Trainium2 NeuronCores.

Strategy
--------
Shard the memory bank (mem_descriptors, rank) across the 8 cores along N.
Host prep (cheap, O(N*F)): normalize rows of both matrices, zero out rows with
rank<=0, pad each 62500-row shard to 65536, transpose to [F, N_loc], and cast
to fp8-e4m3 in the DoubleRow-packed layout [128, 2, N] (k and k+128 paired).

Device (per core): score matrix S = x_hat^T-block @ y_hat tiles on the
TensorEngine (fp8 DoubleRow: one matmul per 512 columns does the full K=256
contraction at ~1.4x bf16 FLOP rate), accumulated in fp32 PSUM as [128, 2048]
four-bank groups.  Each group is max-folded into [128, 2048] running-max
accumulators (acc[j] = max over columns n == j mod 2048).  Fold work is split:
VectorE folds ~1/4 of the groups directly (fp32 from PSUM), ScalarE copies the
other ~3/4 to bf16 SBUF (with a +1.0 bias) and VectorE folds those at its 2x
bf16 rate.  Each accumulator is then folded 2048->512 and finished with one
top-8 max + max_index (positions are column mod 512).

Host finish (tiny): expand each near-max folded position into its candidate
columns, rescore those few candidates exactly in fp32 against the original
data, and emit (cos_max, argmax index, gathered descriptor) matching the
reference semantics.  Correct because the true winner's device score is within
~7e-3 of the device max (fp8 rounding, measured on the real inputs), inside
the 0.02 margin, and every row's global max is positive (so zeroed
invalid/padded entries never win).
"""

import numpy as np
import ml_dtypes

M = 512          # queries
F = 256          # feature dim
N = 500000       # memory rows
NCORES = 8
N_PER = N // NCORES          # 62500
N_PAD = 65536                # per-core padded columns
ACC_W = 2048                 # bf16 accumulator width
IDX_W = 512                  # final fold width (positions are mod IDX_W)
N_BLOCKS = 16                # n-blocks of 4096 columns
BLK = 4096
GW = 2048                    # psum group width (4 banks)
PT = 512                     # single matmul output width
SPLIT = 1536                 # cols [0:SPLIT] drain via ACT, rest via DVE direct
MT = M // 128                # 4 query blocks
KT = F // 128                # 2 contraction tiles (packed into DoubleRow pairs)
MARGIN = 0.02
BIAS = 1.0

_CACHE = {}


def _build_bass():
    import concourse.mybir as mybir
    import concourse.tile as tile
    from concourse import bacc

    fp8 = mybir.dt.float8e4
    bf16 = mybir.dt.bfloat16
    f32 = mybir.dt.float32
    u32 = mybir.dt.uint32
    DR = mybir.MatmulPerfMode.DoubleRow

    nc = bacc.Bacc("TRN2", target_bir_lowering=False, debug=False,
                   num_devices=NCORES)

    xt = nc.dram_tensor("xt", [128, KT, M], fp8, kind="ExternalInput").ap()
    yt = nc.dram_tensor("yt", [128, KT, N_PAD], fp8, kind="ExternalInput").ap()
    # 16 = top-8 from the fp32 acc + top-8 from the bf16 acc
    vals = nc.dram_tensor("vals", [M, 16], f32, kind="ExternalOutput").ap()
    idxs = nc.dram_tensor("idxs", [M, 16], u32, kind="ExternalOutput").ap()

    with tile.TileContext(nc) as tc:
        with (
            tc.tile_pool(name="xpool", bufs=1) as xpool,
            tc.tile_pool(name="ypool", bufs=3) as ypool,
            tc.tile_pool(name="apool", bufs=1) as apool,
            tc.tile_pool(name="spool", bufs=4) as spool,
            tc.tile_pool(name="opool", bufs=4) as opool,
            tc.tile_pool(name="psum", bufs=2, space="PSUM") as pspool,
        ):
            xk = xpool.tile([128, KT, M], fp8, tag="x")
            nc.sync.dma_start(out=xk[:], in_=xt[:])

            acc_v, acc_g = [], []
            for m in range(MT):
                av = apool.tile([128, IDX_W], f32, tag=f"accv{m}")
                nc.gpsimd.memset(av[:], -1e30)
                acc_v.append(av)
                ag = apool.tile([128, ACC_W], bf16, tag=f"accg{m}")
                nc.gpsimd.memset(ag[:], 0.0)   # biased scores are all >= 0.7
                acc_g.append(ag)

            for b in range(N_BLOCKS):
                yk = ypool.tile([128, KT, BLK], fp8, tag="y")
                nc.sync.dma_start(out=yk[:],
                                  in_=yt[:, :, b * BLK:(b + 1) * BLK])
                for m in range(MT):
                    for grp in range(BLK // GW):
                        ps = pspool.tile([128, GW], f32, tag="ps")
                        for sub in range(GW // PT):
                            c0 = grp * GW + sub * PT
                            nc.tensor.matmul(
                                ps[:, sub * PT:(sub + 1) * PT],
                                xk[:, :, m * 128:(m + 1) * 128],
                                yk[:, :, c0:c0 + PT],
                                start=True, stop=True, perf_mode=DR)
                        # split drain: ACT takes [0:SPLIT], DVE the rest
                        stg = spool.tile([128, SPLIT], bf16, tag="stg")
                        nc.scalar.add(stg[:], ps[:, 0:SPLIT], BIAS)
                        av = acc_v[m][:]
                        nc.vector.tensor_max(av, av, ps[:, SPLIT:GW])
                        ag = acc_g[m][:, 0:SPLIT]
                        nc.vector.tensor_max(ag, ag, stg[:])

            for m in range(MT):
                # acc_v is already IDX_W wide; top-8 directly
                t8 = opool.tile([128, 8], f32, tag="t8")
                i8 = opool.tile([128, 8], u32, tag="i8")
                nc.vector.max(t8[:], acc_v[m][:])
                nc.vector.max_index(i8[:], t8[:], acc_v[m][:])
                nc.sync.dma_start(out=vals[m * 128:(m + 1) * 128, 0:8],
                                  in_=t8[:])
                nc.sync.dma_start(out=idxs[m * 128:(m + 1) * 128, 0:8],
                                  in_=i8[:])

                fg = opool.tile([128, 1024], bf16, tag="fg")
                nc.vector.tensor_max(fg[:], acc_g[m][:, 0:1024],
                                     acc_g[m][:, 1024:2048])
                nc.vector.tensor_max(fg[:, 0:512], fg[:, 0:512],
                                     fg[:, 512:1024])
                t8g = opool.tile([128, 8], bf16, tag="t8g")
                t8gf = opool.tile([128, 8], f32, tag="t8gf")
                i8g = opool.tile([128, 8], u32, tag="i8g")
                nc.vector.max(t8g[:], fg[:, 0:IDX_W])
                nc.vector.max_index(i8g[:], t8g[:], fg[:, 0:IDX_W])
                # un-bias while converting bf16 -> f32
                nc.vector.tensor_scalar_add(t8gf[:], t8g[:], -BIAS)
                nc.sync.dma_start(out=vals[m * 128:(m + 1) * 128, 8:16],
                                  in_=t8gf[:])
                nc.sync.dma_start(out=idxs[m * 128:(m + 1) * 128, 8:16],
                                  in_=i8g[:])

    nc.compile()
    return nc


def _get_nc():
    if "nc" not in _CACHE:
        _CACHE["nc"] = _build_bass()
    return _CACHE["nc"]


def run_device(in_maps, trace=False):
    """Run the compiled SPMD kernel; returns (results, BassKernelResults)."""
    from concourse.bass_utils import run_bass_kernel_spmd
    nc = _get_nc()
    r = run_bass_kernel_spmd(nc, in_maps, core_ids=list(range(NCORES)),
                             trace=trace)
    return r.results, r


def prep_inputs(descriptors, mem_descriptors, rank):
    """Host-side normalization / sharding / layout prep."""
    x = np.asarray(descriptors, np.float32)
    y = np.asarray(mem_descriptors, np.float32)
    r = np.asarray(rank, np.float32)

    xx = np.linalg.norm(x, axis=1, keepdims=True)          # [M,1]
    yy = np.linalg.norm(y, axis=1)                          # [N]
    xhat = (x / np.maximum(xx, 1e-30)).astype(np.float32)
    scale = np.where(r > 0, 1.0 / np.maximum(yy, 1e-30), 0.0).astype(np.float32)

    fp8 = ml_dtypes.float8_e4m3
    # DoubleRow layout: [p, j, col] = mat[col, 128*j + p]
    xt = np.ascontiguousarray(
        xhat.T.reshape(KT, 128, M).transpose(1, 0, 2)).astype(fp8)

    in_maps = []
    for c in range(NCORES):
        sh = y[c * N_PER:(c + 1) * N_PER]                   # [N_PER, F]
        sc = scale[c * N_PER:(c + 1) * N_PER]
        ytc = np.zeros((128, KT, N_PAD), dtype=fp8)
        yh = (sh * sc[:, None]).T                           # [F, N_PER] f32
        ytc[:, :, :N_PER] = yh.reshape(KT, 128, N_PER).transpose(1, 0, 2).astype(fp8)
        in_maps.append({"xt": xt, "yt": ytc})
    return in_maps, x, y, r, xx[:, 0], yy


def finish(results, x, y, r, xx, yy):
    """Expand device candidates, rescore exactly, emit reference-equal output."""
    vals = np.stack([np.asarray(results[c]["vals"], np.float32)
                     for c in range(NCORES)])               # [8, M, 16]
    idxs = np.stack([np.asarray(results[c]["idxs"], np.int64)
                     for c in range(NCORES)])               # [8, M, 16]

    gmax = vals.max(axis=(0, 2))                            # [M]
    keep = vals >= (gmax[None, :, None] - MARGIN)           # [8, M, 16]

    slices = np.arange(N_PAD // IDX_W) * IDX_W              # fold slices

    cos_max = np.empty(M, np.float32)
    best_idx = np.empty(M, np.int64)
    valid = r > 0
    for m in range(M):
        cs, ss = np.nonzero(keep[:, m, :])
        cand = (idxs[cs, m, ss][:, None] + slices[None, :]).ravel()
        cores = np.repeat(cs, slices.size)
        ok = cand < N_PER
        ng = cores[ok] * N_PER + cand[ok]
        ng = np.unique(ng)
        ng = ng[valid[ng]]
        xy = y[ng] @ x[m]                                   # fp32 exact
        cos = xy / np.maximum(xx[m] * yy[ng], np.float32(1e-7))
        j = int(np.argmax(cos))
        ties = np.nonzero(cos == cos[j])[0]
        j = int(ties[np.argmin(ng[ties])])
        cos_max[m] = cos[j]
        best_idx[m] = ng[j]

    out_desc = y[best_idx]
    return (cos_max.astype(np.float32), best_idx.astype(np.int32),
            out_desc.astype(np.float32))


def kernel(descriptors, mem_descriptors, rank):
    in_maps, x, y, r, xx, yy = prep_inputs(descriptors, mem_descriptors, rank)
    results, _ = run_device(in_maps)
    return finish(results, x, y, r, xx, yy)


# revision 20
# speedup vs baseline: 1.1502x; 1.0238x over previous
"""Sharded kNN (cosine retrieval) kernel for 8 Trainium2 NeuronCores.

Strategy
--------
Shard the memory bank (mem_descriptors, rank) across the 8 cores along N.
Host prep (cheap, O(N*F)): normalize rows of both matrices, zero out rows with
rank<=0, pad each 62500-row shard to 65536, transpose to [F, N_loc], and cast
to fp8-e4m3 in the DoubleRow-packed layout [128, 2, N] (k and k+128 paired).

Device (per core): score matrix S = x_hat^T-block @ y_hat tiles on the
TensorEngine (fp8 DoubleRow: one matmul per 512 columns does the full K=256
contraction at ~1.4x bf16 FLOP rate), accumulated in fp32 PSUM as [128, 2048]
four-bank groups.  Each group is max-folded into [128, 2048] running-max
accumulators (acc[j] = max over columns n == j mod 2048).  Fold work is split:
VectorE folds ~1/4 of the groups directly (fp32 from PSUM), ScalarE copies the
other ~3/4 to bf16 SBUF (with a +1.0 bias) and VectorE folds those at its 2x
bf16 rate.  Each accumulator is then folded 2048->512 and finished with one
top-8 max + max_index (positions are column mod 512).

Host finish (tiny): expand each near-max folded position into its candidate
columns, rescore those few candidates exactly in fp32 against the original
data, and emit (cos_max, argmax index, gathered descriptor) matching the
reference semantics.  Correct because the true winner's device score is within
~7e-3 of the device max (fp8 rounding, measured on the real inputs), inside
the 0.02 margin, and every row's global max is positive (so zeroed
invalid/padded entries never win).
"""

import numpy as np
import ml_dtypes

M = 512          # queries
F = 256          # feature dim
N = 500000       # memory rows
NCORES = 8
N_PER = N // NCORES          # 62500
N_PAD = 63488                # per-core padded columns (31 x 2048)
ACC_W = 2048                 # bf16 accumulator width
IDX_W = 512                  # final fold width (positions are mod IDX_W)
N_BLOCKS = 15                # full n-blocks of 4096 columns (+ one half block)
BLK = 4096
GW = 2048                    # psum group width (4 banks)
PT = 512                     # single matmul output width
SPLIT = 1536                 # cols [0:SPLIT] drain via ACT, rest via DVE direct
MT = M // 128                # 4 query blocks
KT = F // 128                # 2 contraction tiles (packed into DoubleRow pairs)
MARGIN = 0.02
BIAS = 1.0

_CACHE = {}


def _build_bass():
    import concourse.mybir as mybir
    import concourse.tile as tile
    from concourse import bacc

    fp8 = mybir.dt.float8e4
    bf16 = mybir.dt.bfloat16
    f32 = mybir.dt.float32
    u32 = mybir.dt.uint32
    DR = mybir.MatmulPerfMode.DoubleRow

    nc = bacc.Bacc("TRN2", target_bir_lowering=False, debug=False,
                   num_devices=NCORES)

    xt = nc.dram_tensor("xt", [128, KT, M], fp8, kind="ExternalInput").ap()
    yt = nc.dram_tensor("yt", [128, KT, N_PAD], fp8, kind="ExternalInput").ap()
    # 16 = top-8 from the fp32 acc + top-8 from the bf16 acc
    vals = nc.dram_tensor("vals", [M, 16], f32, kind="ExternalOutput").ap()
    idxs = nc.dram_tensor("idxs", [M, 16], u32, kind="ExternalOutput").ap()

    with tile.TileContext(nc) as tc:
        with (
            tc.tile_pool(name="xpool", bufs=1) as xpool,
            tc.tile_pool(name="ypool", bufs=3) as ypool,
            tc.tile_pool(name="apool", bufs=1) as apool,
            tc.tile_pool(name="spool", bufs=4) as spool,
            tc.tile_pool(name="opool", bufs=4) as opool,
            tc.tile_pool(name="psum", bufs=2, space="PSUM") as pspool,
        ):
            xk = xpool.tile([128, KT, M], fp8, tag="x")
            nc.sync.dma_start(out=xk[:], in_=xt[:])

            acc_v, acc_g = [], []
            for m in range(MT):
                av = apool.tile([128, IDX_W], f32, tag=f"accv{m}")
                nc.gpsimd.memset(av[:], -1e30)
                acc_v.append(av)
                ag = apool.tile([128, ACC_W], bf16, tag=f"accg{m}")
                nc.gpsimd.memset(ag[:], 0.0)   # biased scores are all >= 0.7
                acc_g.append(ag)

            # HAM warm-up: a burst of dummy matmuls during the initial y DMA
            # keeps the PE busy long enough to lift the clock gate to 8/8
            # before real work starts.  Scores land in a scratch psum tile and
            # are consumed by a TT-min into acc_v[0] (a no-op vs -1e30).
            wps = pspool.tile([128, GW], f32, tag="ps")
            for w in range(12):
                nc.tensor.matmul(wps[:, 0:PT], xk[:, :, 0:128],
                                 xk[:, :, 0:M], start=True, stop=True,
                                 perf_mode=DR)
            nc.vector.tensor_tensor(acc_v[0][:], acc_v[0][:],
                                    wps[:, 0:IDX_W], mybir.AluOpType.min)

            def do_group(ps_src, m):
                # split drain: ACT takes [0:SPLIT], DVE the rest
                stg = spool.tile([128, SPLIT], bf16, tag="stg")
                nc.scalar.add(stg[:], ps_src[:, 0:SPLIT], BIAS)
                av = acc_v[m][:]
                nc.vector.tensor_max(av, av, ps_src[:, SPLIT:GW])
                ag = acc_g[m][:, 0:SPLIT]
                nc.vector.tensor_max(ag, ag, stg[:])

            for b in range(N_BLOCKS + 1):
                bw = BLK if b < N_BLOCKS else (N_PAD - N_BLOCKS * BLK)
                yk = ypool.tile([128, KT, BLK], fp8, tag="y")
                if b == 0:
                    # split the first block's DMA so matmuls can start sooner
                    for h in range(4):
                        nc.sync.dma_start(
                            out=yk[:, :, h * 1024:(h + 1) * 1024],
                            in_=yt[:, :, h * 1024:(h + 1) * 1024])
                else:
                    nc.sync.dma_start(
                        out=yk[:, :, 0:bw],
                        in_=yt[:, :, b * BLK:b * BLK + bw])
                for m in range(MT):
                    for grp in range(bw // GW):
                        ps = pspool.tile([128, GW], f32, tag="ps")
                        for sub in range(GW // PT):
                            c0 = grp * GW + sub * PT
                            nc.tensor.matmul(
                                ps[:, sub * PT:(sub + 1) * PT],
                                xk[:, :, m * 128:(m + 1) * 128],
                                yk[:, :, c0:c0 + PT],
                                start=True, stop=True, perf_mode=DR)
                        do_group(ps, m)

            for m in range(MT):
                # acc_v is already IDX_W wide; top-8 directly
                t8 = opool.tile([128, 8], f32, tag="t8")
                i8 = opool.tile([128, 8], u32, tag="i8")
                nc.vector.max(t8[:], acc_v[m][:])
                nc.vector.max_index(i8[:], t8[:], acc_v[m][:])
                nc.sync.dma_start(out=vals[m * 128:(m + 1) * 128, 0:8],
                                  in_=t8[:])
                nc.sync.dma_start(out=idxs[m * 128:(m + 1) * 128, 0:8],
                                  in_=i8[:])

                fg = opool.tile([128, 1024], bf16, tag="fg")
                nc.vector.tensor_max(fg[:], acc_g[m][:, 0:1024],
                                     acc_g[m][:, 1024:2048])
                nc.vector.tensor_max(fg[:, 0:512], fg[:, 0:512],
                                     fg[:, 512:1024])
                t8g = opool.tile([128, 8], bf16, tag="t8g")
                t8gf = opool.tile([128, 8], f32, tag="t8gf")
                i8g = opool.tile([128, 8], u32, tag="i8g")
                nc.vector.max(t8g[:], fg[:, 0:IDX_W])
                nc.vector.max_index(i8g[:], t8g[:], fg[:, 0:IDX_W])
                # un-bias while converting bf16 -> f32
                nc.vector.tensor_scalar_add(t8gf[:], t8g[:], -BIAS)
                nc.sync.dma_start(out=vals[m * 128:(m + 1) * 128, 8:16],
                                  in_=t8gf[:])
                nc.sync.dma_start(out=idxs[m * 128:(m + 1) * 128, 8:16],
                                  in_=i8g[:])

    nc.compile()
    return nc


def _get_nc():
    if "nc" not in _CACHE:
        _CACHE["nc"] = _build_bass()
    return _CACHE["nc"]


def run_device(in_maps, trace=False):
    """Run the compiled SPMD kernel; returns (results, BassKernelResults)."""
    from concourse.bass_utils import run_bass_kernel_spmd
    nc = _get_nc()
    r = run_bass_kernel_spmd(nc, in_maps, core_ids=list(range(NCORES)),
                             trace=trace)
    return r.results, r


def prep_inputs(descriptors, mem_descriptors, rank):
    """Host-side normalization / sharding / layout prep."""
    x = np.asarray(descriptors, np.float32)
    y = np.asarray(mem_descriptors, np.float32)
    r = np.asarray(rank, np.float32)

    xx = np.linalg.norm(x, axis=1, keepdims=True)          # [M,1]
    yy = np.linalg.norm(y, axis=1)                          # [N]
    xhat = (x / np.maximum(xx, 1e-30)).astype(np.float32)
    scale = np.where(r > 0, 1.0 / np.maximum(yy, 1e-30), 0.0).astype(np.float32)

    fp8 = ml_dtypes.float8_e4m3
    # DoubleRow layout: [p, j, col] = mat[col, 128*j + p]
    xt = np.ascontiguousarray(
        xhat.T.reshape(KT, 128, M).transpose(1, 0, 2)).astype(fp8)

    in_maps = []
    for c in range(NCORES):
        sh = y[c * N_PER:(c + 1) * N_PER]                   # [N_PER, F]
        sc = scale[c * N_PER:(c + 1) * N_PER]
        ytc = np.zeros((128, KT, N_PAD), dtype=fp8)
        yh = (sh * sc[:, None]).T                           # [F, N_PER] f32
        ytc[:, :, :N_PER] = yh.reshape(KT, 128, N_PER).transpose(1, 0, 2).astype(fp8)
        in_maps.append({"xt": xt, "yt": ytc})
    return in_maps, x, y, r, xx[:, 0], yy


def finish(results, x, y, r, xx, yy):
    """Expand device candidates, rescore exactly, emit reference-equal output."""
    vals = np.stack([np.asarray(results[c]["vals"], np.float32)
                     for c in range(NCORES)])               # [8, M, 16]
    idxs = np.stack([np.asarray(results[c]["idxs"], np.int64)
                     for c in range(NCORES)])               # [8, M, 16]

    gmax = vals.max(axis=(0, 2))                            # [M]
    keep = vals >= (gmax[None, :, None] - MARGIN)           # [8, M, 16]

    slices = np.arange(N_PAD // IDX_W) * IDX_W              # fold slices

    cos_max = np.empty(M, np.float32)
    best_idx = np.empty(M, np.int64)
    valid = r > 0
    for m in range(M):
        cs, ss = np.nonzero(keep[:, m, :])
        cand = (idxs[cs, m, ss][:, None] + slices[None, :]).ravel()
        cores = np.repeat(cs, slices.size)
        ok = cand < N_PER
        ng = cores[ok] * N_PER + cand[ok]
        ng = np.unique(ng)
        ng = ng[valid[ng]]
        xy = y[ng] @ x[m]                                   # fp32 exact
        cos = xy / np.maximum(xx[m] * yy[ng], np.float32(1e-7))
        j = int(np.argmax(cos))
        ties = np.nonzero(cos == cos[j])[0]
        j = int(ties[np.argmin(ng[ties])])
        cos_max[m] = cos[j]
        best_idx[m] = ng[j]

    out_desc = y[best_idx]
    return (cos_max.astype(np.float32), best_idx.astype(np.int32),
            out_desc.astype(np.float32))


def kernel(descriptors, mem_descriptors, rank):
    in_maps, x, y, r, xx, yy = prep_inputs(descriptors, mem_descriptors, rank)
    results, _ = run_device(in_maps)
    return finish(results, x, y, r, xx, yy)
